# revision 61
# baseline (speedup 1.0000x reference)
"""Trainium2 Bass kernel for nn_EuclideanDistance (retrieval_knn).

out = quantize(x_pad) @ quantize(temp)
  where temp  = [weight; broadcast(bias, L rows)],  bias = colsum(weight^2)/L
        x_pad = [x, ones(B, L)]
        quantize(t) = round(t/s)*s,  s = max(max|t|/127, 1e-12)  (per tensor)

Strategy: shard the stored-vector axis N=16384 across 8 cores (2048 each),
replicate x. Per-tensor scales sx, sw are global scalars computed on host.

Numerics: round(t/s) are integers |k| <= 127, exact in bf16; the integer
matmul accumulates exactly in f32 PSUM (|sum| <= 544*127^2 < 2^24), so the
bf16 PE matmul reproduces the reference fp32 computation to ~1e-5.

The kernel computes out^T (N on partitions): lhsT = quantized weight chunks,
rhs = quantized x^T. In this orientation the contribution of the L ones
columns x the bias rows --- sum_l k1*kb[n] = L*k1*kb[n], constant across B ---
is a per-partition scalar, folded for free into the PSUM-evacuate op
(out = (psum + c) * sx*sw). That removes the ragged 5th K-chunk: K = 4x128.
"""

import sys
import time

import numpy as np

try:
    import concourse.bacc as bacc  # noqa: F401
except ImportError:  # fresh interpreter without the repo on sys.path
    sys.path.insert(0, "/opt/trn_rl_repo")

import concourse.bacc as bacc
import concourse.mybir as mybir
import concourse.tile as tile
from concourse import bass_utils

B, D, N = 1024, 512, 16384
NCORES = 8
NS = N // NCORES          # 2048 stored vectors per core
L = 32                    # split_square_len
QMAX = np.float32(127.0)  # 2**(8-1) - 1
MAGIC = 12582912.0        # 1.5 * 2**23: float32 round-to-nearest-even trick
KC = D // 128             # 4 K-chunks
NC = NS // 128            # 16 output-partition chunks
BT = B // 512             # 2 rhs tiles

F32 = mybir.dt.float32
BF16 = mybir.dt.bfloat16
I8 = mybir.dt.int8

_NC_CACHE = None


def _body(nc, tc, xT, w8, sc, cb, outT):
    from contextlib import ExitStack

    ID = mybir.ActivationFunctionType.Identity
    ADD = mybir.AluOpType.add
    MULT = mybir.AluOpType.mult

    with ExitStack() as ctx:
        cpool = ctx.enter_context(tc.tile_pool(name="const", bufs=1))
        qpool = ctx.enter_context(tc.tile_pool(name="qk", bufs=1))
        spool = ctx.enter_context(tc.tile_pool(name="stage", bufs=3))
        ppool = ctx.enter_context(tc.tile_pool(name="psum", bufs=8, space="PSUM"))
        opool = ctx.enter_context(tc.tile_pool(name="osb", bufs=4))

        scv = cpool.tile([128, 4], F32, name="scv")
        nc.sync.dma_start(scv, sc)
        inv_sx = scv[:, 0:1]
        inv_sw = scv[:, 1:2]
        sxsw = scv[:, 2:3]
        magic = scv[:, 3:4]
        cbv = cpool.tile([128, 2 * NC], F32, name="cbv")

        # ---- loads, all on the sync HWDGE ring (the scalar ring measures
        #      ~2.5x slower). Strict FIFO, so: first-x-half and the first
        #      w8 chunk lead (shortest path to the first matmul), stores
        #      trail every load. w8 is int8, 1 MB total. ----
        xfs = []
        wfs = []
        for k in range(KC):
            xf = spool.tile([128, B], F32, name="xf", tag="xf", bufs=4)
            xfs.append(xf)
            wf = spool.tile([128, NS], I8, name="wf", tag=f"wf{k}", bufs=1)
            wfs.append(wf)
        # head of each w8 chunk (cols 0:512, 64 KB) is all phase A needs;
        # the 192 KB tails stream after every x byte is in flight
        for k in range(KC):
            r = slice(k * 128, (k + 1) * 128)
            nc.sync.dma_start(xfs[k][:, 0:512], xT[r, 0:512])
            nc.sync.dma_start(wfs[k][:, 0:512], w8[r, 0:512])
            nc.sync.dma_start(xfs[k][:, 512:B], xT[r, 512:B])
        nc.sync.dma_start(cbv, cb)   # needed only by the evacs
        for k in range(KC):
            r = slice(k * 128, (k + 1) * 128)
            nc.sync.dma_start(wfs[k][:, 512:NS], w8[r, 512:NS])

        # ---- PE warm-up: dummy matmuls on a memset tile run during the
        #      (PE-idle) input fill and trip the HAM clock gate to 8/8,
        #      so the real matmuls start at 2.4 GHz ----
        wrm = spool.tile([128, 640], BF16, name="wrm", bufs=1)
        nc.vector.memset(wrm, 0.0)
        ps_warm = ppool.tile([128, B], F32, name="ps", tag="ps", bufs=4)
        for _ in range(19):
            nc.tensor.matmul(ps_warm[:, 0:512], wrm[:, 0:128],
                             wrm[:, 128:640], start=True, stop=True)

        # ---- quantize x (device) / convert w int8 -> bf16 ----
        kxs = []
        kws = []
        for k in range(KC):
            kw = qpool.tile([128, NS], BF16, name=f"kw{k}", tag=f"kw{k}")
            nc.vector.tensor_copy(kw[:, 0:512], wfs[k][:, 0:512])
            kws.append(kw)

            xm = spool.tile([128, B], F32, name="xm", tag="xm", bufs=4)
            kx = qpool.tile([128, B], BF16, name=f"kx{k}", tag=f"kx{k}")
            for h in range(2):  # halves, matching the split loads
                hs = slice(h * 512, (h + 1) * 512)
                nc.scalar.activation(xm[:, hs], xfs[k][:, hs], ID,
                                     bias=magic, scale=inv_sx)
                nc.vector.tensor_scalar_add(kx[:, hs], xm[:, hs], -MAGIC)
            kxs.append(kx)

        # ---- 16 output chunks, paired into 1 MB stores ----
        def evac(j, ps, obs, on_dve):
            if on_dve:
                # (psum + c_int) * (sx*sw) on DVE
                nc.vector.tensor_scalar(obs, ps, cbv[:, j:j + 1],
                                        sxsw, ADD, MULT)
            else:
                # psum * (sx*sw) + c_scaled on ACT
                nc.scalar.activation(obs, ps, ID,
                                     bias=cbv[:, NC + j:NC + j + 1],
                                     scale=sxsw)

        def store_pair(jp, ob):
            j0 = jp * 2
            # one 1 MB store for both 128-row chunks: fewer DMA
            # completions on the ring. Pairs 2 and 5 ride the (slow but
            # idle) scalar ring, so the sync ring has no backlog left to
            # drain after the final evacuation.
            eng = nc.scalar if jp in (2, 5) else nc.sync
            eng.dma_start(
                outT[j0 * 128:(j0 + 2) * 128, :]
                .rearrange("(a p) c -> p a c", p=128),
                ob.rearrange("p (a c) -> p a c", a=2))

        # Phase A: the first 4 groups k-major, so PE has 24 issueable
        # matmuls (k<3) while the tail x chunks are still in flight --- a
        # j-major order stalls the PE FIFO at j0/k3 behind kx3's DMA.
        psA = [ppool.tile([128, B], F32, name="ps", tag="ps", bufs=4)
               for _ in range(4)]
        obA = [opool.tile([128, 2 * B], F32, name="ob", tag="ob", bufs=6)
               for _ in range(2)]
        for k in range(KC):
            if k < KC - 1:
                order = [(b, j) for b in range(BT) for j in range(4)]
            else:  # close groups j-major so j0's psum frees before A ends
                order = [(b, j) for j in range(4) for b in range(BT)]
            for b, j in order:
                lhsT = kws[k][:, j * 128:(j + 1) * 128]
                nc.tensor.matmul(
                    psA[j][:, b * 512:(b + 1) * 512], lhsT,
                    kxs[k][:, b * 512:(b + 1) * 512],
                    start=(k == 0), stop=(k == KC - 1))
        for j in range(4):
            evac(j, psA[j], obA[j // 2][:, (j % 2) * B:(j % 2 + 1) * B],
                 on_dve=(j % 2 == 0))
            if j % 2 == 1:
                store_pair(j // 2, obA[j // 2])

        # w8 tail casts AFTER phase A's evacs in the DVE FIFO: they are
        # needed only by phase B, and ahead of the evacs they stall the
        # PSUM-slot release (measured ~1.5us of PE gaps at the A->B seam)
        for k in range(KC):
            nc.vector.tensor_copy(kws[k][:, 512:NS], wfs[k][:, 512:NS])

        # Phase B: remaining groups j-major (all inputs resident by now).
        # The final pair stores per-j with its evacs split across both
        # engines --- minimizes the post-last-matmul drain tail.
        for jp in range(2, NC // 2):
            last = jp >= NC // 2 - 2   # fine-grained stores for last 2 pairs
            ob = opool.tile([128, 2 * B], F32, name="ob", tag="ob", bufs=6)
            for h in range(2):
                j = jp * 2 + h
                ps = ppool.tile([128, B], F32, name="ps", tag="ps", bufs=4)
                for k in range(KC):
                    lhsT = kws[k][:, j * 128:(j + 1) * 128]
                    for b in range(BT):
                        nc.tensor.matmul(
                            ps[:, b * 512:(b + 1) * 512], lhsT,
                            kxs[k][:, b * 512:(b + 1) * 512],
                            start=(k == 0), stop=(k == KC - 1))
                obs = ob[:, h * B:(h + 1) * B]
                if not last:
                    evac(j, ps, obs, on_dve=(h == 0))
                else:
                    # split each evac over DVE+ACT and store per 256 KB half
                    # the moment its evac lands: the final drain then waits
                    # only on the ACT half's small store
                    nc.vector.tensor_scalar(obs[:, 0:512], ps[:, 0:512],
                                            cbv[:, j:j + 1], sxsw,
                                            ADD, MULT)
                    nc.sync.dma_start(outT[j * 128:(j + 1) * 128, 0:512],
                                      obs[:, 0:512])
                    nc.scalar.activation(obs[:, 512:B], ps[:, 512:B], ID,
                                         bias=cbv[:, NC + j:NC + j + 1],
                                         scale=sxsw)
                    nc.sync.dma_start(outT[j * 128:(j + 1) * 128, 512:B],
                                      obs[:, 512:B])
            if not last:
                store_pair(jp, ob)


def _build():
    global _NC_CACHE
    if _NC_CACHE is not None:
        return _NC_CACHE
    nc = bacc.Bacc("TRN2", target_bir_lowering=False, debug=False,
                   enable_asserts=False, num_devices=1)
    xT = nc.dram_tensor("xT", [D, B], F32, kind="ExternalInput").ap()
    w8 = nc.dram_tensor("w8", [D, NS], I8, kind="ExternalInput").ap()
    sc = nc.dram_tensor("sc", [128, 4], F32, kind="ExternalInput").ap()
    cb = nc.dram_tensor("cb", [128, 2 * NC], F32, kind="ExternalInput").ap()
    outT = nc.dram_tensor("outT", [NS, B], F32, kind="ExternalOutput").ap()
    with tile.TileContext(nc) as tc:
        _body(nc, tc, xT, w8, sc, cb, outT)
    nc.compile()
    _NC_CACHE = nc
    return nc


def _prepare_inputs(x, weight, split_square_len):
    assert x.shape == (B, D) and weight.shape == (D, N)
    assert int(split_square_len) == L

    x = np.ascontiguousarray(x, dtype=np.float32)
    weight = np.ascontiguousarray(weight, dtype=np.float32)

    # bias = colsum(weight^2)/L in f32, matching the reference
    bias = (np.einsum("dn,dn->n", weight, weight, dtype=np.float32)
            / np.float32(L)).astype(np.float32)

    # global per-tensor scales (f32 arithmetic to match jax)
    max_x = np.float32(max(np.abs(x).max(), np.float32(1.0)))
    sx = np.maximum(max_x / QMAX, np.float32(1e-12))
    max_w = np.float32(max(np.abs(weight).max(), np.abs(bias).max()))
    sw = np.maximum(max_w / QMAX, np.float32(1e-12))

    x_T = np.ascontiguousarray(x.T)  # [D, B]

    sc = np.zeros((128, 4), dtype=np.float32)
    sc[:, 0] = np.float32(1.0) / sx
    sc[:, 1] = np.float32(1.0) / sw
    sc[:, 2] = sx * sw
    sc[:, 3] = np.float32(MAGIC)

    # ones/bias rank-1 term: c[n] = L * round(1/sx) * round(bias[n]/sw),
    # exact integers; divides (not reciprocal-mults) to match the reference.
    k1 = np.float32(np.round(np.float32(1.0) / sx))
    kb = np.round(bias / sw).astype(np.float32)
    c_int = (np.float32(L) * k1) * kb          # exact in f32 (< 2^24)
    c_scaled = c_int * (sx * sw)

    # stored-vector database, quantized offline (true divide = reference)
    w_q = np.round(weight / sw).astype(np.int8)

    in_maps = []
    for c in range(NCORES):
        sl = slice(c * NS, (c + 1) * NS)
        cb = np.concatenate([
            c_int[sl].reshape(NC, 128).T,      # [128, NC], col j = chunk j
            c_scaled[sl].reshape(NC, 128).T,
        ], axis=1).astype(np.float32)
        cb = np.ascontiguousarray(cb)
        in_maps.append({
            "xT": x_T,
            "w8": np.ascontiguousarray(w_q[:, sl]),
            "sc": sc,
            "cb": cb,
        })
    return in_maps


def _run(in_maps, **kwargs):
    nc = _build()
    return bass_utils.run_bass_kernel_spmd(
        nc, in_maps, core_ids=list(range(NCORES)), **kwargs)


def kernel(x, weight, split_square_len):
    in_maps = _prepare_inputs(x, weight, split_square_len)
    res = None
    for attempt in range(3):
        try:
            res = _run(in_maps)
            break
        except Exception:
            # transient NRT_EXEC_UNIT_UNRECOVERABLE device wedges have been
            # observed on this fabric; a clean re-execute recovers
            if attempt == 2:
                raise
            time.sleep(2.0)
    outT = np.concatenate([res.results[c]["outT"] for c in range(NCORES)],
                          axis=0)          # [N, B]
    return outT.T                          # [B, N] view
